# revision 3
# baseline (speedup 1.0000x reference)
"""Trainium2 Bass kernel for nn_DecompMultiTransform (RGCN basis-decomposition).

Reference computation:
    full_w = (w_comp @ weight).reshape(64, 256, 256)   # per-type weights
    out[n, :] = x[n, :] @ full_w[xtype[n]]             # N = 4096

Scheme (type-parallel, minimal FLOPs):
  Host sorts rows by type into 64 zero-padded groups of CAP rows (pure
  layout - permutation, padding, transpose, bf16 cast). Core c owns types
  8c..8c+7. On device:

  Stage 1 - build this core's 8 per-type weight matrices on the PE:
      W_tau[i, o] = sum_b w_comp[tau, b] * weight[b, i*256+o]
    The contraction K packs (r=8 o-columns x b=16 bases) = 128 so the PE
    runs full-K matmuls:  lhsT = wstack_g[(r,b), j]  (a host re-layout of
    weight), rhs = cdelta[(r,b), (r', t)] which holds w_comp values
    delta-masked on r==r'. 64 matmuls of [K=128, M=128, N=64] produce
    W_tau[i, o] tiles with i on partitions; strided copies move banks
    PSUM->SBUF as bf16. Bank-pair 2q,2q+1 completes o-quarter q (both
    i-halves).

  Stage 2 - per type, per o-quarter: out_t[n, 64q:64q+64] accumulated
    over the two i-halves with x stationary (lhsT = xsT[i, n]). The
    o-quarter granularity lets stage-2 chase stage-1 chunk arrivals, and
    output halves drain (PSUM->SBUF copy -> DMA) as soon as both their
    quarters finish, so only the last o-quarter + one pair-drain chain
    trails the final weight chunk.

  DMA: inputs split over both HWDGE rings (sync: W quarters 0,2 + last
  bank; scalar: cdelta, x halves, W quarter 1 + bank 6) so both rings
  stream from the start of the program; outputs ride the same rings
  behind the inputs. All operands bf16 (PSUM accumulates f32). Host
  un-sorts the output.
"""

import sys

if "/opt/trn_rl_repo" not in sys.path:
    sys.path.insert(0, "/opt/trn_rl_repo")

import numpy as np

import concourse.bass as bass
import concourse.mybir as mybir
import concourse.tile as tile
from concourse import bacc
from concourse.bass_utils import run_bass_kernel_spmd

P = 128
N_FULL = 4096
IN_DIM = 256
OUT_DIM = 256
NUM_B = 16
NUM_T = 64
N_CORES = 8
TPC = NUM_T // N_CORES            # 8 types per core
NG = 64                           # stage-1 groups: (ih 2) x (og 32)
G_PER_BANK = 8                    # one PSUM bank = 8 groups

F32 = mybir.dt.float32
BF16 = mybir.dt.bfloat16
NP_BF16 = mybir.dt.np(BF16)

# weight DMA chunks: (group start, size, ring) -- banks 0-1, 2-3, 4-5, 6, 7
W_CHUNKS = ((0, 16, "sync"), (16, 16, "scalar"), (32, 16, "sync"),
            (48, 8, "scalar"), (56, 8, "sync"))


def _build_program(cap):
    nc = bacc.Bacc("TRN2", target_bir_lowering=False, debug=False)

    # xsT6[j, h, ih, tt, n]: x value for type t=4h+tt, row-slot n, input
    # i = ih*128 + j. Each h-half is contiguous per partition.
    xsT = nc.declare_dram_parameter("xsT", [P, 2, 2, TPC // 2, cap], BF16,
                                    isOutput=False)
    cdelta = nc.declare_dram_parameter("cdelta", [P, 8 * TPC], BF16,
                                       isOutput=False)
    wstack = nc.declare_dram_parameter("wstack", [P, NG, P], BF16,
                                       isOutput=False)
    # outb[h, pair, n, u, oc]: out for type t=2*pair+u, row n, o=h*128+oc
    outb = nc.declare_dram_parameter("outb", [2, TPC // 2, cap, 2, P], BF16,
                                     isOutput=True)

    with tile.TileContext(nc) as tc:
        with (
            tc.tile_pool(name="const", bufs=1) as constp,
            tc.tile_pool(name="wpool", bufs=1) as wpool,
            tc.tile_pool(name="wsbp", bufs=1) as wsbp,
            tc.tile_pool(name="stp", bufs=4) as stp,
            tc.tile_pool(name="ps1", bufs=3, space="PSUM") as ps1,
            tc.tile_pool(name="pso", bufs=1, space="PSUM") as pso,
        ):
            cd = constp.tile([P, 8 * TPC], BF16, name="cd")
            xst = constp.tile([P, 2, 2, TPC // 2, cap], BF16, name="xst")
            wchunks = []
            for g0, sz, ring in W_CHUNKS:
                wt = wpool.tile([P, sz, P], BF16, name=f"w{g0}")
                wchunks.append((g0, sz, wt))

            # ---- input DMA triggers. Both rings start immediately; ring
            # loads balanced so weight chunks land in program order.
            nc.scalar.dma_start(out=cd[:], in_=cdelta.ap()[:, :])
            nc.sync.dma_start(out=wchunks[0][2][:],
                              in_=wstack.ap()[:, 0:16, :])
            nc.scalar.dma_start(out=xst[:, 0], in_=xsT.ap()[:, 0])
            nc.scalar.dma_start(out=xst[:, 1], in_=xsT.ap()[:, 1])
            nc.sync.dma_start(out=wchunks[2][2][:],
                              in_=wstack.ap()[:, 32:48, :])
            nc.scalar.dma_start(out=wchunks[1][2][:],
                                in_=wstack.ap()[:, 16:32, :])
            nc.scalar.dma_start(out=wchunks[3][2][:],
                                in_=wstack.ap()[:, 48:56, :])
            nc.sync.dma_start(out=wchunks[4][2][:],
                              in_=wstack.ap()[:, 56:64, :])

            def wslice(s):
                for gs, sz, wt in wchunks:
                    if gs <= s < gs + sz:
                        return wt[:, s - gs, :]
                raise AssertionError(s)

            wsb = [
                wsbp.tile([P, TPC, OUT_DIM], BF16, name=f"wsb{ih}")
                for ih in range(2)
            ]
            # one PSUM bank per type pair: po[pair][n, u, o]
            pos = [
                pso.tile([cap, 2, OUT_DIM], F32, name=f"po{i}", space="PSUM")
                for i in range(TPC // 2)
            ]

            def s1_bank(b):
                ps = ps1.tile([P, G_PER_BANK, 8 * TPC], F32, name="ps1",
                              tag="ps1", space="PSUM")
                for k in range(G_PER_BANK):
                    nc.tensor.matmul(out=ps[:, k, :], lhsT=wslice(b * 8 + k),
                                     rhs=cd[:], start=True, stop=True)
                # scatter bank into wsb[ih][:, t, o-quarter] (bf16)
                ih, q = b % 2, b // 2
                src = ps[:].rearrange("p gl (rp t) -> p t gl rp", rp=8, t=TPC)
                dst = wsb[ih][:][:, :, q * 64:(q + 1) * 64].rearrange(
                    "p t (gl rp) -> p t gl rp", gl=G_PER_BANK, rp=8)
                return src, dst

            def xs(ih, t):
                return xst[:, t // 4, ih, t % 4, :]

            def s2_mm(t, q, ih):
                nc.tensor.matmul(
                    out=pos[t // 2][:, t % 2, q * 64:(q + 1) * 64],
                    lhsT=xs(ih, t),
                    rhs=wsb[ih][:, t, q * 64:(q + 1) * 64],
                    start=(ih == 0),
                    stop=(ih == 1),
                )

            def drain(pr, h, ceng, deng):
                st = stp.tile([cap, 2, P], BF16, name="st", tag="st")
                src = pos[pr][:, :, h * P:(h + 1) * P]
                if ceng == "v":
                    nc.vector.tensor_copy(out=st[:], in_=src)
                else:
                    nc.scalar.copy(st[:], src)
                deng.dma_start(out=outb.ap()[h, pr], in_=st)

            # ---- banks 0-5 with stage-2 quarters 0-1 and A-half drains
            for b in range(4):
                src, dst = s1_bank(b)
                if b % 2 == 0:
                    nc.vector.tensor_copy(out=dst, in_=src)
                else:
                    nc.scalar.copy(dst, src)
                if b % 2 == 1:
                    q = b // 2
                    for t in range(TPC):
                        s2_mm(t, q, 0)
                        s2_mm(t, q, 1)
            # A-half (o 0:128) complete for all types: drain 4 pairs
            drain(0, 0, "v", nc.sync)
            drain(1, 0, "a", nc.scalar)
            drain(2, 0, "v", nc.sync)
            drain(3, 0, "a", nc.scalar)

            for b in (4, 5):
                src, dst = s1_bank(b)
                if b % 2 == 0:
                    nc.vector.tensor_copy(out=dst, in_=src)
                else:
                    nc.scalar.copy(dst, src)
            for t in range(TPC):
                s2_mm(t, 2, 0)
                s2_mm(t, 2, 1)

            # ---- endgame: banks 6,7 -> q3 pair-major with B-half drains
            # riding right behind each pair (only one accumulation group
            # may be open per PSUM bank, so ih0/ih1 stay adjacent per type)
            src, dst = s1_bank(6)
            nc.vector.tensor_copy(out=dst, in_=src)
            src, dst = s1_bank(7)
            nc.scalar.copy(dst, src)
            for pr in range(TPC // 2):
                for t in (2 * pr, 2 * pr + 1):
                    s2_mm(t, 3, 0)
                    s2_mm(t, 3, 1)
                drain(pr, 1, "v", nc.sync if pr % 2 == 0 else nc.scalar)

    nc.compile()
    return nc


_PROGRAMS = {}
LAST_RESULT = None  # test harness introspection


def kernel(x, xtype, weight, w_comp, trace=False):
    global LAST_RESULT
    x = np.asarray(x, dtype=np.float32)
    xtype = np.asarray(xtype).astype(np.int64)
    weight = np.asarray(weight, dtype=np.float32)
    w_comp = np.asarray(w_comp, dtype=np.float32)
    assert x.shape == (N_FULL, IN_DIM) and weight.shape == (NUM_B, IN_DIM * OUT_DIM)

    # ---- host-side layout: sort rows by type into padded slots ----
    counts = np.bincount(xtype, minlength=NUM_T)
    cap = int(-(-max(counts.max(), 32) // 8) * 8)
    if cap > P:
        raise RuntimeError(f"type count {counts.max()} exceeds {P}")
    if cap not in _PROGRAMS:
        _PROGRAMS[cap] = _build_program(cap)
    nc = _PROGRAMS[cap]

    order = np.argsort(xtype, kind="stable")
    sorted_t = xtype[order]
    starts = np.zeros(NUM_T, dtype=np.int64)
    starts[1:] = np.cumsum(counts)[:-1]
    rank = np.arange(N_FULL, dtype=np.int64) - starts[sorted_t]
    slot = sorted_t * cap + rank  # global padded slot per sorted row

    xpad = np.zeros((NUM_T * cap, IN_DIM), np.float32)
    xpad[slot] = x[order]
    xpad = xpad.astype(NP_BF16)

    # wstack[(r,b), g, j] = weight[b, (ih*128+j)*256 + og*8+r], with the
    # g axis permuted into bank order: bank b = (o-quarter b//2, ih b%2),
    # slot k = og within quarter.
    w5 = weight.reshape(NUM_B, 2, P, 32, 8)  # b, ih, j, og, r
    wst_nat = np.ascontiguousarray(w5.transpose(4, 0, 1, 3, 2)).reshape(P, NG, P)
    perm = np.empty(NG, np.int64)
    for s in range(NG):
        b, k = divmod(s, G_PER_BANK)
        ih, og = b % 2, (b // 2) * 8 + k
        perm[s] = ih * 32 + og
    wstack = np.ascontiguousarray(wst_nat[:, perm, :]).astype(NP_BF16)

    c_bf = w_comp.astype(NP_BF16)

    in_maps = []
    for c in range(N_CORES):
        xc = xpad[c * TPC * cap:(c + 1) * TPC * cap]  # [8*cap, 256] bf16
        # [j, h, ih, tt, n] with t = 4h + tt
        xsT = np.ascontiguousarray(
            xc.reshape(2, TPC // 2, cap, 2, P).transpose(4, 0, 3, 1, 2)
        )
        cdl = np.zeros((8, NUM_B, 8, TPC), NP_BF16)  # r, b, rp, t
        for r in range(8):
            cdl[r, :, r, :] = c_bf[c * TPC:(c + 1) * TPC, :].T
        in_maps.append(
            {
                "xsT": xsT,
                "cdelta": cdl.reshape(P, 8 * TPC),
                "wstack": wstack,
            }
        )

    res = run_bass_kernel_spmd(nc, in_maps, list(range(N_CORES)), trace=trace)
    LAST_RESULT = res

    out = np.empty((N_FULL, OUT_DIM), np.float32)
    for c in range(N_CORES):
        # [h, pair, n, u, oc] -> [pair, u, n, h, oc] -> [8*cap, 256]
        big = res.results[c]["outb"].transpose(1, 3, 2, 0, 4).reshape(
            TPC * cap, OUT_DIM)
        sel = (slot >= c * TPC * cap) & (slot < (c + 1) * TPC * cap)
        out[order[sel]] = big[slot[sel] - c * TPC * cap].astype(np.float32)
    return out
